# revision 14
# baseline (speedup 1.0000x reference)
"""Trainium2 Bass kernel for a 3-layer GRU encoder (B=64, T=1024, D=40, H=256).

Strategy:
  - Data-parallel over batch: 8 cores x B_local=8 (weights replicated).
  - Per core: 3-layer GRU wavefront. Layer l processes timestep s-l at
    superstep s, so all three layers run concurrently on one core.
  - Hidden-major layout everywhere: state h^T is [128 part(hidden), 2 chunk, B].
    All matmuls are weight-stationary (lhsT = fp16 weight tiles, FWL-eligible),
    activations stream as the moving operand (N = B_local).  This avoids all
    per-step transposes.
  - PSUM accumulates fp32; ScalarE applies sigmoid/tanh straight from PSUM;
    VectorE does r*h and the gated blend in fp16 (2x mode).
  - Pointwise conv (kernel_size=1) is composed into layer-0 weights on host:
    W0eff = conv_w @ W0.  Biases enter via a ones-row appended to x
    (b0eff = conv_b @ W0 + b0); b1/b2 get K=1 ones matmuls only if nonzero.

Outputs are returned as (encoder_outs [B,T,256] f32, h0, h1, h2 [B,256] f32),
matching the reference.
"""

import os
from contextlib import ExitStack

import numpy as np

import concourse.bass as bass
import concourse.mybir as mybir
import concourse.tile as tile
from concourse import bacc
from concourse.bass_utils import run_bass_kernel_spmd

H = 256
HC = 2          # hidden chunks of 128
D = 40
NCORES = 8
F16 = mybir.dt.float16
F32 = mybir.dt.float32

SIG = mybir.ActivationFunctionType.Sigmoid
TANH = mybir.ActivationFunctionType.Tanh

# set by run() when tracing
LAST_EXEC_NS = None
LAST_NC = None
_BUILD_CACHE = {}


def build_gru(T, B, nonzero_b12):
    """Build the Bass program for one core (B = per-core batch)."""
    nc = bacc.Bacc("TRN2", target_bir_lowering=False, debug=False)

    # ---- DRAM parameters (declaration order = binding order) ----
    xT = nc.declare_dram_parameter("xT", [D + 1, T, B], F16, isOutput=False)
    w0x_d = nc.declare_dram_parameter("w0x", [D + 1, 3 * H], F16, isOutput=False)
    u0_d = nc.declare_dram_parameter("u0", [128, HC, 3 * H], F16, isOutput=False)
    w1_d = nc.declare_dram_parameter("w1", [128, HC, 3 * H], F16, isOutput=False)
    u1_d = nc.declare_dram_parameter("u1", [128, HC, 3 * H], F16, isOutput=False)
    w2_d = nc.declare_dram_parameter("w2", [128, HC, 3 * H], F16, isOutput=False)
    u2_d = nc.declare_dram_parameter("u2", [128, HC, 3 * H], F16, isOutput=False)
    if nonzero_b12:
        b12_d = nc.declare_dram_parameter("b12", [1, 2, 3 * H], F16, isOutput=False)

    enc_d = nc.declare_dram_parameter("enc", [128, T, HC, B], F16, isOutput=True)
    h0_d = nc.declare_dram_parameter("h0f", [128, HC, B], F16, isOutput=True)
    h1_d = nc.declare_dram_parameter("h1f", [128, HC, B], F16, isOutput=True)

    with tile.TileContext(nc) as tc, ExitStack() as ctx:
        const = ctx.enter_context(tc.tile_pool(name="const", bufs=1))
        work = ctx.enter_context(tc.tile_pool(name="work", bufs=4))
        psum = ctx.enter_context(tc.tile_pool(name="psum", bufs=6, space="PSUM"))

        # ---- load constants into SBUF ----
        x_sb = const.tile([D + 1, T, B], F16, tag="x_sb")
        nc.sync.dma_start(out=x_sb, in_=xT[:, :, :])

        w0x = const.tile([D + 1, 3 * H], F16, tag="w0x")
        nc.sync.dma_start(out=w0x, in_=w0x_d[:, :])
        u0 = const.tile([128, HC, 3 * H], F16, tag="u0")
        nc.sync.dma_start(out=u0, in_=u0_d[:, :, :])
        w1 = const.tile([128, HC, 3 * H], F16, tag="w1")
        nc.sync.dma_start(out=w1, in_=w1_d[:, :, :])
        u1 = const.tile([128, HC, 3 * H], F16, tag="u1")
        nc.sync.dma_start(out=u1, in_=u1_d[:, :, :])
        w2 = const.tile([128, HC, 3 * H], F16, tag="w2")
        nc.sync.dma_start(out=w2, in_=w2_d[:, :, :])
        u2 = const.tile([128, HC, 3 * H], F16, tag="u2")
        nc.sync.dma_start(out=u2, in_=u2_d[:, :, :])

        if nonzero_b12:
            b12 = const.tile([1, 2, 3 * H], F16, tag="b12")
            nc.sync.dma_start(out=b12, in_=b12_d[:, :, :])
            ones1 = const.tile([1, B], F16, tag="ones1")
            nc.vector.memset(ones1, 1.0)

        # persistent state (ping-pong per superstep parity)
        h0_buf = [const.tile([128, HC, B], F16, tag=f"h0_{i}", name=f"h0_{i}")
                  for i in range(2)]
        h1_buf = [const.tile([128, HC, B], F16, tag=f"h1_{i}", name=f"h1_{i}")
                  for i in range(2)]
        h2z = const.tile([128, HC, B], F16, tag="h2z")
        for t_ in (*h0_buf, *h1_buf, h2z):
            nc.vector.memset(t_, 0.0)

        # full output sequence of layer 2 lives in SBUF (hidden-major)
        ring = const.tile([128, T, HC, B], F16, tag="ring")

        OUT_BLK = min(128, T)  # DMA the encoder output every OUT_BLK steps
        assert T % OUT_BLK == 0

        # per-layer weight chunk tables: list of (lhsT chunk AP [K,768], kind)
        # kind: 'x' (layer-0 input), 'below' (h of layer below), 'self', 'rh'
        def in_chunks(l):
            if l == 0:
                return [(w0x, "x")]
            w = w1 if l == 1 else w2
            return [(w[:, c, :], ("below", c)) for c in range(HC)]

        def self_chunks(l):
            u = (u0, u1, u2)[l]
            return [(u[:, c, :], ("self", c)) for c in range(HC)]

        def rhs_of(kind, l, s):
            if kind == "x":
                return x_sb[:, s, :]
            what, c = kind
            if what == "below":
                if l == 1:
                    return h0_buf[(s - 1) % 2][:, c, :]
                # l == 2: below is h1
                return h1_buf[(s - 1) % 2][:, c, :]
            raise AssertionError

        def self_rhs(l, s, t):
            # own previous state for layer l at its local time t (= value
            # written at superstep s-1)
            if l == 0:
                return lambda c: h0_buf[(s - 1) % 2][:, c, :]
            if l == 1:
                return lambda c: h1_buf[(s - 1) % 2][:, c, :]
            if t == 0:
                return lambda c: h2z[:, c, :]
            return lambda c: ring[:, t - 1, c, :]

        total = T + 2
        for s in range(total):
            for l in range(3):
                t = s - l  # layer-local timestep
                if t < 0 or t >= T:
                    continue

                ics = in_chunks(l)
                scs = self_chunks(l)
                srhs = self_rhs(l, s, t)

                ps = psum.tile([128, 6, B], F32, tag="ps")

                # --- zr matmuls: r tiles (m=2,3) first, then z (m=0,1) ---
                for m in (2, 3, 0, 1):
                    seq = []
                    for w, kind in ics:
                        seq.append((w[:, m * 128:(m + 1) * 128], rhs_of(kind, l, s)))
                    for u, kind in scs:
                        seq.append((u[:, m * 128:(m + 1) * 128], srhs(kind[1])))
                    if nonzero_b12 and l >= 1:
                        seq.append((b12[:, l - 1, m * 128:(m + 1) * 128], ones1))
                    n = len(seq)
                    for i, (lhsT, rhs) in enumerate(seq):
                        nc.tensor.matmul(
                            ps[:, m, :], lhsT, rhs,
                            start=(i == 0), stop=(i == n - 1),
                        )

                # --- gates ---
                r = work.tile([128, HC, B], F16, tag="r")
                nc.scalar.activation(r, ps[:, 2:4, :], SIG)
                z = work.tile([128, HC, B], F16, tag="z")
                nc.scalar.activation(z, ps[:, 0:2, :], SIG)

                # previous own state as one AP for DVE ops
                if l == 0:
                    hp_ap = h0_buf[(s - 1) % 2]
                elif l == 1:
                    hp_ap = h1_buf[(s - 1) % 2]
                else:
                    hp_ap = h2z if t == 0 else ring[:, t - 1, :, :]

                rh = work.tile([128, HC, B], F16, tag="rh")
                nc.vector.tensor_mul(rh, r, hp_ap)

                # --- h-gate matmuls: per M-tile one complete group
                # (input-part, then rh-part; only one group open per bank) ---
                for m in (4, 5):
                    seq = []
                    for w, kind in ics:
                        seq.append((w[:, m * 128:(m + 1) * 128],
                                    rhs_of(kind, l, s)))
                    for c in range(HC):
                        seq.append((scs[c][0][:, m * 128:(m + 1) * 128],
                                    rh[:, c, :]))
                    if nonzero_b12 and l >= 1:
                        seq.append((b12[:, l - 1, m * 128:(m + 1) * 128], ones1))
                    n = len(seq)
                    for i, (lhsT, rhs) in enumerate(seq):
                        nc.tensor.matmul(
                            ps[:, m, :], lhsT, rhs,
                            start=(i == 0), stop=(i == n - 1),
                        )

                hh = work.tile([128, HC, B], F16, tag="hh")
                nc.scalar.activation(hh, ps[:, 4:6, :], TANH)

                # --- blend: h' = hh + z * (hprev - hh) ---
                d = work.tile([128, HC, B], F16, tag="d")
                nc.vector.tensor_sub(d, hp_ap, hh)
                zd = work.tile([128, HC, B], F16, tag="zd")
                nc.vector.tensor_mul(zd, z, d)

                if l == 0:
                    hnew = h0_buf[s % 2]
                elif l == 1:
                    hnew = h1_buf[s % 2]
                else:
                    hnew = ring[:, t, :, :]
                nc.vector.tensor_add(hnew, hh, zd)

                # --- stream encoder output out per block ---
                if l == 2 and (t + 1) % OUT_BLK == 0:
                    blk = t + 1 - OUT_BLK
                    nc.sync.dma_start(
                        out=enc_d[:, blk:t + 1, :, :],
                        in_=ring[:, blk:t + 1, :, :],
                    )

        # final states of layers 0 and 1
        nc.sync.dma_start(out=h0_d[:, :, :], in_=h0_buf[(T - 1) % 2])
        nc.sync.dma_start(out=h1_d[:, :, :], in_=h1_buf[T % 2])

    nc.compile()
    return nc


def _prep_inputs(encoder_inputs, conv_w, conv_b, Ws, Us, bs):
    """Host-side weight prep. Returns per-core input maps' shared tensors."""
    B, T, D_ = encoder_inputs.shape
    # compose conv into layer-0 input projection
    W0eff = conv_w.astype(np.float64) @ Ws[0].astype(np.float64)  # [D, 3H]
    b0eff = conv_b.astype(np.float64) @ Ws[0].astype(np.float64) + bs[0]

    w0x = np.concatenate([W0eff, b0eff[None, :]], axis=0)  # [D+1, 3H]

    def chunk(u):  # [256, 3H] -> [128, 2, 3H]
        return np.stack([u[:128], u[128:]], axis=1)

    tensors = {
        "w0x": w0x.astype(np.float16),
        "u0": chunk(Us[0]).astype(np.float16),
        "w1": chunk(Ws[1]).astype(np.float16),
        "u1": chunk(Us[1]).astype(np.float16),
        "w2": chunk(Ws[2]).astype(np.float16),
        "u2": chunk(Us[2]).astype(np.float16),
    }
    nonzero_b12 = bool(np.any(bs[1] != 0) or np.any(bs[2] != 0))
    if nonzero_b12:
        tensors["b12"] = np.stack([bs[1], bs[2]])[None].astype(np.float16)
    return tensors, nonzero_b12


def kernel(encoder_inputs, conv_w, conv_b, W0, U0, b0, W1, U1, b1, W2, U2, b2,
           trace=False):
    global LAST_EXEC_NS
    encoder_inputs = np.asarray(encoder_inputs, dtype=np.float32)
    B, T, D_ = encoder_inputs.shape
    assert D_ == D
    assert B % NCORES == 0
    Bl = B // NCORES

    tensors, nonzero_b12 = _prep_inputs(
        encoder_inputs, np.asarray(conv_w), np.asarray(conv_b),
        [np.asarray(W0), np.asarray(W1), np.asarray(W2)],
        [np.asarray(U0), np.asarray(U1), np.asarray(U2)],
        [np.asarray(b0), np.asarray(b1), np.asarray(b2)],
    )

    key = (T, Bl, nonzero_b12)
    if key not in _BUILD_CACHE:
        _BUILD_CACHE[key] = build_gru(T, Bl, nonzero_b12)
    nc = _BUILD_CACHE[key]

    # x transposed + ones row: [D+1, T, Bl] per core
    xT_full = encoder_inputs.transpose(2, 1, 0)  # [D, T, B]
    in_maps = []
    for c in range(NCORES):
        xs = xT_full[:, :, c * Bl:(c + 1) * Bl]
        xs = np.concatenate([xs, np.ones((1, T, Bl), np.float32)], axis=0)
        m = {"xT": np.ascontiguousarray(xs).astype(np.float16)}
        m.update(tensors)
        in_maps.append(m)

    global LAST_NC
    LAST_NC = nc
    try:
        res = run_bass_kernel_spmd(
            nc, in_maps, core_ids=list(range(NCORES)), trace=trace,
        )
    except (ImportError, ModuleNotFoundError):
        res = run_bass_kernel_spmd(
            nc, in_maps, core_ids=list(range(NCORES)), trace=False,
        )
    LAST_EXEC_NS = res.exec_time_ns

    # ---- gather / unshard ----
    enc = np.empty((B, T, H), dtype=np.float32)
    h0 = np.empty((B, H), dtype=np.float32)
    h1 = np.empty((B, H), dtype=np.float32)
    for c in range(NCORES):
        r = res.results[c]
        e = r["enc"].astype(np.float32)          # [128, T, HC, Bl]
        # -> [Bl, T, HC*128]
        e = e.transpose(3, 1, 2, 0).reshape(Bl, T, H)
        enc[c * Bl:(c + 1) * Bl] = e
        h0[c * Bl:(c + 1) * Bl] = (
            r["h0f"].astype(np.float32).transpose(2, 1, 0).reshape(Bl, H)
        )
        h1[c * Bl:(c + 1) * Bl] = (
            r["h1f"].astype(np.float32).transpose(2, 1, 0).reshape(Bl, H)
        )
    h2 = enc[:, -1, :].copy()
    return enc, h0, h1, h2


# revision 16
# speedup vs baseline: 55.2338x; 55.2338x over previous
"""Trainium2 Bass kernel for a 3-layer GRU encoder (B=64, T=1024, D=40, H=256).

Strategy:
  - Data-parallel over batch: 8 cores x B_local=8 (weights replicated).
  - Per core: 3-layer GRU wavefront. Layer l processes timestep s-l at
    superstep s, so all three layers run concurrently on one core.
  - Hidden-major layout everywhere: state h^T is [128 part(hidden), 2 chunk, B].
    All matmuls are weight-stationary (lhsT = fp16 weight tiles, FWL-eligible),
    activations stream as the moving operand (N = B_local).  This avoids all
    per-step transposes.
  - PSUM accumulates fp32; ScalarE applies sigmoid/tanh straight from PSUM;
    VectorE does r*h and the gated blend in fp16 (2x mode).
  - Pointwise conv (kernel_size=1) is composed into layer-0 weights on host:
    W0eff = conv_w @ W0.  Biases enter via a ones-row appended to x
    (b0eff = conv_b @ W0 + b0); b1/b2 get K=1 ones matmuls only if nonzero.

Outputs are returned as (encoder_outs [B,T,256] f32, h0, h1, h2 [B,256] f32),
matching the reference.
"""

import os
from contextlib import ExitStack

import numpy as np

import concourse.bass as bass
import concourse.mybir as mybir
import concourse.tile as tile
from concourse import bacc
from concourse.bass_utils import run_bass_kernel_spmd

H = 256
HC = 2          # hidden chunks of 128
D = 40
NCORES = 8
F16 = mybir.dt.float16
F32 = mybir.dt.float32

SIG = mybir.ActivationFunctionType.Sigmoid
TANH = mybir.ActivationFunctionType.Tanh

# set by run() when tracing
LAST_EXEC_NS = None
LAST_NC = None
_BUILD_CACHE = {}


def build_gru(T, B, nonzero_b12):
    """Build the Bass program for one core (B = per-core batch)."""
    nc = bacc.Bacc("TRN2", target_bir_lowering=False, debug=False)

    # ---- DRAM parameters (declaration order = binding order) ----
    xT = nc.declare_dram_parameter("xT", [D + 1, T, B], F16, isOutput=False)
    w0x_d = nc.declare_dram_parameter("w0x", [D + 1, 3 * H], F16, isOutput=False)
    u0_d = nc.declare_dram_parameter("u0", [128, HC, 3 * H], F16, isOutput=False)
    w1_d = nc.declare_dram_parameter("w1", [128, HC, 3 * H], F16, isOutput=False)
    u1_d = nc.declare_dram_parameter("u1", [128, HC, 3 * H], F16, isOutput=False)
    w2_d = nc.declare_dram_parameter("w2", [128, HC, 3 * H], F16, isOutput=False)
    u2_d = nc.declare_dram_parameter("u2", [128, HC, 3 * H], F16, isOutput=False)
    if nonzero_b12:
        b12_d = nc.declare_dram_parameter("b12", [1, 2, 3 * H], F16, isOutput=False)

    enc_d = nc.declare_dram_parameter("enc", [128, T, HC, B], F16, isOutput=True)
    h0_d = nc.declare_dram_parameter("h0f", [128, HC, B], F16, isOutput=True)
    h1_d = nc.declare_dram_parameter("h1f", [128, HC, B], F16, isOutput=True)

    with tile.TileContext(nc) as tc, ExitStack() as ctx:
        const = ctx.enter_context(tc.tile_pool(name="const", bufs=1))
        work = ctx.enter_context(tc.tile_pool(name="work", bufs=4))
        psum = ctx.enter_context(tc.tile_pool(name="psum", bufs=6, space="PSUM"))

        # ---- load constants into SBUF ----
        x_sb = const.tile([D + 1, T, B], F16, tag="x_sb")
        nc.sync.dma_start(out=x_sb, in_=xT[:, :, :])

        w0x = const.tile([D + 1, 3 * H], F16, tag="w0x")
        nc.sync.dma_start(out=w0x, in_=w0x_d[:, :])
        u0 = const.tile([128, HC, 3 * H], F16, tag="u0")
        nc.sync.dma_start(out=u0, in_=u0_d[:, :, :])
        w1 = const.tile([128, HC, 3 * H], F16, tag="w1")
        nc.sync.dma_start(out=w1, in_=w1_d[:, :, :])
        u1 = const.tile([128, HC, 3 * H], F16, tag="u1")
        nc.sync.dma_start(out=u1, in_=u1_d[:, :, :])
        w2 = const.tile([128, HC, 3 * H], F16, tag="w2")
        nc.sync.dma_start(out=w2, in_=w2_d[:, :, :])
        u2 = const.tile([128, HC, 3 * H], F16, tag="u2")
        nc.sync.dma_start(out=u2, in_=u2_d[:, :, :])

        if nonzero_b12:
            b12 = const.tile([1, 2, 3 * H], F16, tag="b12")
            nc.sync.dma_start(out=b12, in_=b12_d[:, :, :])
            ones1 = const.tile([1, B], F16, tag="ones1")
            nc.vector.memset(ones1, 1.0)

        # persistent state (ping-pong per superstep parity)
        h0_buf = [const.tile([128, HC, B], F16, tag=f"h0_{i}", name=f"h0_{i}")
                  for i in range(2)]
        h1_buf = [const.tile([128, HC, B], F16, tag=f"h1_{i}", name=f"h1_{i}")
                  for i in range(2)]
        h2z = const.tile([128, HC, B], F16, tag="h2z")
        for t_ in (*h0_buf, *h1_buf, h2z):
            nc.vector.memset(t_, 0.0)

        # full output sequence of layer 2 lives in SBUF (hidden-major)
        ring = const.tile([128, T, HC, B], F16, tag="ring")

        OUT_BLK = min(128, T)  # DMA the encoder output every OUT_BLK steps
        assert T % OUT_BLK == 0

        # per-layer weight chunk tables: list of (lhsT chunk AP [K,768], kind)
        # kind: 'x' (layer-0 input), 'below' (h of layer below), 'self', 'rh'
        def in_chunks(l):
            if l == 0:
                return [(w0x, "x")]
            w = w1 if l == 1 else w2
            return [(w[:, c, :], ("below", c)) for c in range(HC)]

        def self_chunks(l):
            u = (u0, u1, u2)[l]
            return [(u[:, c, :], ("self", c)) for c in range(HC)]

        def rhs_of(kind, l, s):
            if kind == "x":
                return x_sb[:, s, :]
            what, c = kind
            if what == "below":
                if l == 1:
                    return h0_buf[(s - 1) % 2][:, c, :]
                # l == 2: below is h1
                return h1_buf[(s - 1) % 2][:, c, :]
            raise AssertionError

        def self_rhs(l, s, t):
            # own previous state for layer l at its local time t (= value
            # written at superstep s-1)
            if l == 0:
                return lambda c: h0_buf[(s - 1) % 2][:, c, :]
            if l == 1:
                return lambda c: h1_buf[(s - 1) % 2][:, c, :]
            if t == 0:
                return lambda c: h2z[:, c, :]
            return lambda c: ring[:, t - 1, c, :]

        total = T + 2
        for s in range(total):
            for l in range(3):
                t = s - l  # layer-local timestep
                if t < 0 or t >= T:
                    continue

                ics = in_chunks(l)
                scs = self_chunks(l)
                srhs = self_rhs(l, s, t)

                ps = psum.tile([128, 6, B], F32, tag="ps")

                # --- zr matmuls: r tiles (m=2,3) first, then z (m=0,1) ---
                for m in (2, 3, 0, 1):
                    seq = []
                    for w, kind in ics:
                        seq.append((w[:, m * 128:(m + 1) * 128], rhs_of(kind, l, s)))
                    for u, kind in scs:
                        seq.append((u[:, m * 128:(m + 1) * 128], srhs(kind[1])))
                    if nonzero_b12 and l >= 1:
                        seq.append((b12[:, l - 1, m * 128:(m + 1) * 128], ones1))
                    n = len(seq)
                    for i, (lhsT, rhs) in enumerate(seq):
                        nc.tensor.matmul(
                            ps[:, m, :], lhsT, rhs,
                            start=(i == 0), stop=(i == n - 1),
                        )

                # --- gates: one sigmoid over z+r (saves an ACT op) ---
                zr = work.tile([128, 4, B], F16, tag="zr")
                nc.scalar.activation(zr, ps[:, 0:4, :], SIG)
                z = zr[:, 0:2, :]
                r = zr[:, 2:4, :]

                # previous own state as one AP for DVE ops
                if l == 0:
                    hp_ap = h0_buf[(s - 1) % 2]
                elif l == 1:
                    hp_ap = h1_buf[(s - 1) % 2]
                else:
                    hp_ap = h2z if t == 0 else ring[:, t - 1, :, :]

                rh = work.tile([128, HC, B], F16, tag="rh")
                nc.vector.tensor_mul(rh, r, hp_ap)

                # --- h-gate matmuls: per M-tile one complete group
                # (input-part, then rh-part; only one group open per bank) ---
                for m in (4, 5):
                    seq = []
                    for w, kind in ics:
                        seq.append((w[:, m * 128:(m + 1) * 128],
                                    rhs_of(kind, l, s)))
                    for c in range(HC):
                        seq.append((scs[c][0][:, m * 128:(m + 1) * 128],
                                    rh[:, c, :]))
                    if nonzero_b12 and l >= 1:
                        seq.append((b12[:, l - 1, m * 128:(m + 1) * 128], ones1))
                    n = len(seq)
                    for i, (lhsT, rhs) in enumerate(seq):
                        nc.tensor.matmul(
                            ps[:, m, :], lhsT, rhs,
                            start=(i == 0), stop=(i == n - 1),
                        )

                # --- blend: h' = z*h + (1-z)*hh, with z*h and (1-z)
                # computed before tanh lands (shorter post-tanh chain) ---
                zh = work.tile([128, HC, B], F16, tag="zh")
                nc.vector.tensor_mul(zh, z, hp_ap)
                z1 = work.tile([128, HC, B], F16, tag="z1")
                nc.vector.tensor_scalar(
                    z1, z, -1.0, 1.0,
                    mybir.AluOpType.mult, mybir.AluOpType.add,
                )

                hh = work.tile([128, HC, B], F16, tag="hh")
                nc.scalar.activation(hh, ps[:, 4:6, :], TANH)

                z1h = work.tile([128, HC, B], F16, tag="z1h")
                nc.vector.tensor_mul(z1h, z1, hh)

                if l == 0:
                    hnew = h0_buf[s % 2]
                elif l == 1:
                    hnew = h1_buf[s % 2]
                else:
                    hnew = ring[:, t, :, :]
                nc.vector.tensor_add(hnew, zh, z1h)

                # --- stream encoder output out per block ---
                if l == 2 and (t + 1) % OUT_BLK == 0:
                    blk = t + 1 - OUT_BLK
                    nc.sync.dma_start(
                        out=enc_d[:, blk:t + 1, :, :],
                        in_=ring[:, blk:t + 1, :, :],
                    )

        # final states of layers 0 and 1
        nc.sync.dma_start(out=h0_d[:, :, :], in_=h0_buf[(T - 1) % 2])
        nc.sync.dma_start(out=h1_d[:, :, :], in_=h1_buf[T % 2])

    nc.compile()
    return nc


def _prep_inputs(encoder_inputs, conv_w, conv_b, Ws, Us, bs):
    """Host-side weight prep. Returns per-core input maps' shared tensors."""
    B, T, D_ = encoder_inputs.shape
    # compose conv into layer-0 input projection
    W0eff = conv_w.astype(np.float64) @ Ws[0].astype(np.float64)  # [D, 3H]
    b0eff = conv_b.astype(np.float64) @ Ws[0].astype(np.float64) + bs[0]

    w0x = np.concatenate([W0eff, b0eff[None, :]], axis=0)  # [D+1, 3H]

    def chunk(u):  # [256, 3H] -> [128, 2, 3H]
        return np.stack([u[:128], u[128:]], axis=1)

    tensors = {
        "w0x": w0x.astype(np.float16),
        "u0": chunk(Us[0]).astype(np.float16),
        "w1": chunk(Ws[1]).astype(np.float16),
        "u1": chunk(Us[1]).astype(np.float16),
        "w2": chunk(Ws[2]).astype(np.float16),
        "u2": chunk(Us[2]).astype(np.float16),
    }
    nonzero_b12 = bool(np.any(bs[1] != 0) or np.any(bs[2] != 0))
    if nonzero_b12:
        tensors["b12"] = np.stack([bs[1], bs[2]])[None].astype(np.float16)
    return tensors, nonzero_b12


def kernel(encoder_inputs, conv_w, conv_b, W0, U0, b0, W1, U1, b1, W2, U2, b2,
           trace=False):
    global LAST_EXEC_NS
    encoder_inputs = np.asarray(encoder_inputs, dtype=np.float32)
    B, T, D_ = encoder_inputs.shape
    assert D_ == D
    assert B % NCORES == 0
    Bl = B // NCORES

    tensors, nonzero_b12 = _prep_inputs(
        encoder_inputs, np.asarray(conv_w), np.asarray(conv_b),
        [np.asarray(W0), np.asarray(W1), np.asarray(W2)],
        [np.asarray(U0), np.asarray(U1), np.asarray(U2)],
        [np.asarray(b0), np.asarray(b1), np.asarray(b2)],
    )

    key = (T, Bl, nonzero_b12)
    if key not in _BUILD_CACHE:
        _BUILD_CACHE[key] = build_gru(T, Bl, nonzero_b12)
    nc = _BUILD_CACHE[key]

    # x transposed + ones row: [D+1, T, Bl] per core
    xT_full = encoder_inputs.transpose(2, 1, 0)  # [D, T, B]
    in_maps = []
    for c in range(NCORES):
        xs = xT_full[:, :, c * Bl:(c + 1) * Bl]
        xs = np.concatenate([xs, np.ones((1, T, Bl), np.float32)], axis=0)
        m = {"xT": np.ascontiguousarray(xs).astype(np.float16)}
        m.update(tensors)
        in_maps.append(m)

    global LAST_NC
    LAST_NC = nc
    try:
        res = run_bass_kernel_spmd(
            nc, in_maps, core_ids=list(range(NCORES)), trace=trace,
        )
    except (ImportError, ModuleNotFoundError):
        res = run_bass_kernel_spmd(
            nc, in_maps, core_ids=list(range(NCORES)), trace=False,
        )
    LAST_EXEC_NS = res.exec_time_ns

    # ---- gather / unshard ----
    enc = np.empty((B, T, H), dtype=np.float32)
    h0 = np.empty((B, H), dtype=np.float32)
    h1 = np.empty((B, H), dtype=np.float32)
    for c in range(NCORES):
        r = res.results[c]
        e = r["enc"].astype(np.float32)          # [128, T, HC, Bl]
        # -> [Bl, T, HC*128]
        e = e.transpose(3, 1, 2, 0).reshape(Bl, T, H)
        enc[c * Bl:(c + 1) * Bl] = e
        h0[c * Bl:(c + 1) * Bl] = (
            r["h0f"].astype(np.float32).transpose(2, 1, 0).reshape(Bl, H)
        )
        h1[c * Bl:(c + 1) * Bl] = (
            r["h1f"].astype(np.float32).transpose(2, 1, 0).reshape(Bl, H)
        )
    h2 = enc[:, -1, :].copy()
    return enc, h0, h1, h2
